# revision 26
# baseline (speedup 1.0000x reference)
import os
import sys

import numpy as np

sys.path.insert(0, "/opt/trn_rl_repo")

# Problem constants (nn_AdditiveAttention): hardcoded per spec.
B, NQ, NK, D, DV, H = 4, 512, 512, 512, 512, 128
NCORES = 8
QPC = NQ // NCORES  # queries contributed by each batch to each core (64)
SMAX = 6144         # max s/t pipeline tile free dim (per partition)
SUBQ = 64           # queries per softmax sub-group
WARM_MM = True      # emit HAM-warmer dummy matmuls

LAST_EXEC_NS = None
LAST_RESULT = {}


def _plan(valid_lens):
    L = [int(x) for x in np.asarray(valid_lens).reshape(-1)]
    L2 = [min(NK, -(-l // 2) * 2) for l in L]       # add/tanh/scores extent
    KPV = [min(NK, -(-l // 128) * 128) for l in L]  # PV (128-aligned) extent
    # Per batch: sub-groups of SUBQ queries, each a list of (chunk, fused).
    # The cheapest batch (smallest L2) is offloaded to GPSIMD as whole-chunk
    # tensor_tensor broadcast adds, if small enough to fit its slow rate.
    GB = -1  # gpsimd TT offload disabled: walrus re-engines it onto DVE
    CH = {}
    for b in range(B):
        c = 16
        while c * L2[b] > SMAX:
            c //= 2
        sgs = []
        for sg in range(QPC // SUBQ):
            specs = []
            left = SUBQ
            while left > 0:
                step = min(c, left)
                specs.append((step, False))
                left -= step
            sgs.append(specs)
        CH[b] = sgs
    return L, L2, KPV, CH, GB


def _build_program(L, L2, KPV, CH, GB):
    """Build the SPMD Bass program. All cores run this one program;
    per-core data differences come only through in_maps."""
    import concourse.bacc as bacc
    import concourse.mybir as mybir
    from concourse.tile import TileContext

    f32 = mybir.dt.float32
    bf16 = mybir.dt.bfloat16
    OFF2 = np.concatenate([[0], np.cumsum(L2)]).astype(int)
    OFFV = np.concatenate([[0], np.cumsum(KPV)]).astype(int)
    KSUM2 = int(OFF2[-1])
    KSUMV = int(OFFV[-1])
    NQL = B * QPC  # local queries per core (256)

    nc = bacc.Bacc("TRN2", target_bir_lowering=False, debug=False)

    qt_d = nc.dram_tensor("qt", [D, NQL], bf16, kind="ExternalInput")
    kt_d = nc.dram_tensor("kt", [D, KSUM2], bf16, kind="ExternalInput")
    v_d = nc.dram_tensor("v", [KSUMV, DV], bf16, kind="ExternalInput")
    wq_d = nc.dram_tensor("wq", [D, H], bf16, kind="ExternalInput")
    wk_d = nc.dram_tensor("wk", [D, H], bf16, kind="ExternalInput")
    oneh_d = nc.dram_tensor("oneh", [H, SUBQ * SUBQ], bf16, kind="ExternalInput")
    eye_d = nc.dram_tensor("eye", [SUBQ, SUBQ], bf16, kind="ExternalInput")
    out_d = nc.dram_tensor("out", [NQL, DV], f32, kind="ExternalOutput")

    Tanh = mybir.ActivationFunctionType.Tanh
    Exp = mybir.ActivationFunctionType.Exp
    Copy = mybir.ActivationFunctionType.Copy
    AX = mybir.AxisListType.X

    with TileContext(nc) as tc:
        with (
            tc.tile_pool(name="const", bufs=1) as cpool,
            tc.tile_pool(name="proj", bufs=1) as projpool,
            tc.tile_pool(name="s", bufs=4) as spool,
            tc.tile_pool(name="t", bufs=4) as tpool,
            tc.tile_pool(name="p", bufs=2) as ppool,
            tc.tile_pool(name="stat", bufs=6) as statpool,
            tc.tile_pool(name="osb", bufs=2) as opool,
        ):
            # ---- load constants (critical-path DMAs first, split across the
            # sync and gpsimd queues; small per-chunk loads are combined into
            # single DMAs; V tiles are loaded later, inside the main loop)
            kt_sb = [cpool.tile([128, KSUM2], bf16, tag=f"kt{i}", name=f"kt{i}") for i in range(4)]
            wkb = cpool.tile([128, 4 * H], bf16, tag="wkb")
            wqb = cpool.tile([128, 4 * H], bf16, tag="wqb")
            qtb = cpool.tile([128, 4 * NQL], bf16, tag="qtb")
            wk_sb = [wkb[:, i * H : (i + 1) * H] for i in range(4)]
            wq_sb = [wqb[:, i * H : (i + 1) * H] for i in range(4)]
            qt_sb = [qtb[:, i * NQL : (i + 1) * NQL] for i in range(4)]
            for i in range(4):
                eng = nc.sync if i % 2 == 0 else nc.gpsimd
                eng.dma_start(kt_sb[i][:], kt_d.rearrange("(n p) m -> n p m", p=128)[i])
            nc.sync.dma_start(wkb[:, :].rearrange("p (n m) -> p n m", n=4), wk_d.rearrange("(n p) m -> p n m", p=128))
            nc.gpsimd.dma_start(qtb[:, :].rearrange("p (n m) -> p n m", n=4), qt_d.rearrange("(n p) m -> p n m", p=128))
            nc.sync.dma_start(wqb[:, :].rearrange("p (n m) -> p n m", n=4), wq_d.rearrange("(n p) m -> p n m", p=128))
            oneh_sb = cpool.tile([128, SUBQ * SUBQ], bf16, tag="oneh")
            nc.gpsimd.dma_start(oneh_sb[:], oneh_d[:])
            eye_sb = cpool.tile([SUBQ, SUBQ], bf16, tag="eye")
            nc.sync.dma_start(eye_sb[:], eye_d[:])
            v_sb = [cpool.tile([128, DV], bf16, tag=f"v{i}", name=f"v{i}") for i in range(KSUMV // 128)]

            def load_v():
                for i in range(KSUMV // 128):
                    nc.sync.dma_start(
                        v_sb[i][:], v_d.rearrange("(n p) m -> n p m", p=128)[i]
                    )

            # ---- projections (bf16 in, f32 psum; QpT f32 / KpT bf16 out)
            qp_sb = projpool.tile([128, NQL], f32, tag="qp")
            kp_sb = [
                projpool.tile(
                    [128, L2[b]], f32 if b == GB else bf16,
                    tag=f"kp{b}", name=f"kp{b}",
                )
                for b in range(B)
            ]
            first_b = next(b for b in range(B) if b != GB)
            border = [first_b, -1] + [b for b in range(B) if b not in (first_b, -1, GB)] + ([GB] if GB >= 0 else [])
            with tc.tile_pool(name="pps", bufs=2, space="PSUM") as projps:
                for pb in border:
                    if pb == -1:
                        qp_ps = projps.tile([128, 512], f32, tag="projps", name="qp_ps")
                        for dc in range(4):
                            nc.tensor.matmul(
                                qp_ps[:, :NQL], wq_sb[dc][:], qt_sb[dc][:],
                                start=(dc == 0), stop=(dc == 3),
                            )
                        nc.scalar.copy(qp_sb[:], qp_ps[:, :NQL])
                        continue
                    b = pb
                    c0 = int(OFF2[b])
                    cw = L2[b]
                    kp_ps = projps.tile([128, 512], f32, tag="projps", name="kp_ps")
                    for dc in range(4):
                        nc.tensor.matmul(
                            kp_ps[:, :cw], wk_sb[dc][:], kt_sb[dc][:, c0 : c0 + cw],
                            start=(dc == 0), stop=(dc == 3),
                        )
                    nc.scalar.copy(kp_sb[b][:, :], kp_ps[:, :cw])

            # ---- main phase: one 32-query sub-group at a time, each with its
            # own PSUM tile; softmax/PV of sub-group g is split into 2 stages
            # drip-fed between later chunks so no engine stalls on the chain.
            with (
                tc.tile_pool(name="sps", bufs=3, space="PSUM") as scorps,
                tc.tile_pool(name="ops", bufs=2, space="PSUM") as ops,
                tc.tile_pool(name="tps", bufs=2, space="PSUM") as tps,
                tc.tile_pool(name="wps", bufs=1, space="PSUM") as wps,
            ):
                warm_ps = wps.tile([1, 8], f32, tag="warm")
                pending = []

                def softmax_stages(b, sg, sc_ps):
                    """Softmax + P@V for sub-group (b, sg) as 2 drip stages."""
                    kpadv = KPV[b]
                    koffv = int(OFFV[b])
                    lb = L[b]
                    r0 = b * QPC + sg * SUBQ  # output row base
                    box = {}

                    def s1():
                        nmx = statpool.tile([128, 1], f32, tag="nmx", name="nmx")
                        nc.vector.reduce_max(
                            nmx[:SUBQ, :], sc_ps[:SUBQ, :lb], axis=AX, negate=True
                        )
                        p_t = ppool.tile([SUBQ, 512], bf16, tag="p", name="p_t")
                        ssum = statpool.tile([128, 1], f32, tag="ssum", name="ssum")
                        nc.scalar.activation(
                            p_t[:, :lb], sc_ps[:SUBQ, :lb], Exp,
                            bias=nmx[:SUBQ, :], accum_out=ssum[:SUBQ, :],
                        )
                        if lb < kpadv:
                            nc.gpsimd.memset(p_t[:, lb:kpadv], 0.0)
                        box["p_t"] = p_t
                        box["ssum"] = ssum

                    def s2():
                        rs = statpool.tile([128, 1], f32, tag="rs", name="rs")
                        nc.vector.reciprocal(rs[:SUBQ, :], box["ssum"][:SUBQ, :])
                        o_ps = ops.tile([SUBQ, DV], f32, tag="ops", name="o_ps")
                        for kc in range(kpadv // 128):
                            wt_ps = tps.tile([128, SUBQ], bf16, tag="wtps", name="wt_ps")
                            nc.tensor.transpose(
                                wt_ps[:],
                                box["p_t"][:, kc * 128 : (kc + 1) * 128],
                                eye_sb[:],
                            )
                            wt_sb = statpool.tile([128, SUBQ], bf16, tag="wtsb", name="wt_sb")
                            nc.vector.tensor_copy(wt_sb[:], wt_ps[:])
                            nc.tensor.matmul(
                                o_ps[:],
                                wt_sb[:],
                                v_sb[(koffv + kc * 128) // 128][:],
                                start=(kc == 0), stop=(kc == kpadv // 128 - 1),
                            )
                        o_sb = opool.tile([SUBQ, DV], f32, tag="osb", name="o_sb")
                        nc.scalar.activation(
                            o_sb[:], o_ps[:], Copy, scale=rs[:SUBQ, :]
                        )
                        nc.sync.dma_start(out_d[r0 : r0 + SUBQ, :], o_sb[:])

                    return [s1, s2]

                # GPSIMD batch: emit all broadcast-add chunks up front;
                # their tanh+matmul consumption blocks drip into the stream.
                GCH = 8  # queries per gpsimd chunk
                g_s = []
                if GB >= 0:
                    gkpad = L2[GB]
                    for c in range(QPC // GCH):
                        s_t = spool.tile(
                            [128, GCH * gkpad], f32, tag=f"gs{c}", name="g_s_t"
                        )
                        kp_b = kp_sb[GB][:, :].rearrange(
                            "p (o k) -> p o k", o=1
                        ).broadcast_to((128, GCH, gkpad))
                        q0 = GB * QPC + c * GCH
                        qp_b = qp_sb[:, q0 : q0 + GCH].rearrange(
                            "p (c o) -> p c o", o=1
                        ).broadcast_to((128, GCH, gkpad))
                        nc.gpsimd.tensor_tensor(
                            out=s_t[:, :].rearrange("p (c k) -> p c k", c=GCH),
                            in0=kp_b, in1=qp_b, op=mybir.AluOpType.add,
                        )
                        g_s.append(s_t)

                g_sc = {}

                def g_block(c):
                    """Consume gpsimd chunk c: tanh + score matmuls."""
                    gkpad = L2[GB]
                    if c == 0:
                        g_sc[0] = scorps.tile(
                            [SUBQ, gkpad], f32, tag="scores", name="sc_ps"
                        )
                    sc_ps = g_sc[0]
                    t_t = tpool.tile([128, SMAX], bf16, tag="t", name="t_t")
                    nc.scalar.activation(
                        t_t[:, : GCH * gkpad], g_s[c][:, :], Tanh
                    )
                    for j in range(GCH):
                        jj = c * GCH + j
                        nc.tensor.matmul(
                            sc_ps[:SUBQ, :gkpad],
                            oneh_sb[:, jj * SUBQ : (jj + 1) * SUBQ],
                            t_t[:, j * gkpad : j * gkpad + gkpad],
                            start=(jj == 0), stop=(jj == SUBQ - 1),
                        )
                    if jj == SUBQ - 1:
                        pending.extend(softmax_stages(GB, 0, sc_ps))

                if GB >= 0:
                    pending.extend(
                        (lambda cc: (lambda: g_block(cc)))(c)
                        for c in range(QPC // GCH)
                    )

                for b in range(B):
                    if b == GB:
                        continue
                    kpad = L2[b]
                    for sg in range(QPC // SUBQ):
                        sc_ps = scorps.tile(
                            [SUBQ, kpad], f32, tag="scores", name="sc_ps"
                        )
                        qbase = 0
                        for chunk, fused in CH[b][sg]:
                            t_t = tpool.tile([128, SMAX], bf16, tag="t", name="t_t")
                            if fused:
                                for j in range(chunk):
                                    q = b * QPC + sg * SUBQ + qbase + j
                                    nc.scalar.activation(
                                        t_t[:, j * kpad : (j + 1) * kpad],
                                        kp_sb[b][:, :], Tanh,
                                        bias=qp_sb[:, q : q + 1],
                                    )
                            else:
                                s_t = spool.tile([128, SMAX], bf16, tag="s", name="s_t")
                                for j in range(chunk):
                                    q = b * QPC + sg * SUBQ + qbase + j
                                    nc.vector.tensor_scalar_add(
                                        s_t[:, j * kpad : (j + 1) * kpad],
                                        kp_sb[b][:, :],
                                        qp_sb[:, q : q + 1],
                                    )
                                if WARM_MM:
                                    nc.tensor.matmul(
                                        warm_ps[:, :1],
                                        oneh_sb[:, 0:1],
                                        s_t[:, 0:1],
                                        start=True, stop=True,
                                    )
                                fd = chunk * kpad
                                nc.scalar.activation(
                                    t_t[:, :fd], s_t[:, :fd], Tanh
                                )
                            for j in range(chunk):
                                jj = sg * SUBQ % SUBQ + qbase + j  # within group
                                nc.tensor.matmul(
                                    sc_ps[:SUBQ, :kpad],
                                    oneh_sb[:, jj * SUBQ : (jj + 1) * SUBQ],
                                    t_t[:, j * kpad : j * kpad + kpad],
                                    start=(jj == 0), stop=(jj == SUBQ - 1),
                                )
                            qbase += chunk
                            if b == (1 if GB != 1 else 2) and sg == 0 and qbase == chunk:
                                load_v()
                            if pending:
                                pending.pop(0)()
                        pending.extend(softmax_stages(b, sg, sc_ps))
                while pending:
                    pending.pop(0)()

    nc.compile()
    return nc


def _install_profile_hook():
    """Register the NTFF profile hook that this container's antenv lacks,
    so run_bass_kernel_spmd(trace=True) can report exec_time_ns."""
    import types

    import antenv

    try:
        import antenv.axon_hooks  # noqa: F401
        return
    except ImportError:
        pass
    try:
        from trn_agent_boot.trn_boot import _ntff_profile_via_ctypes
    except ImportError:
        return
    hook = _ntff_profile_via_ctypes("/opt/axon/libaxon_pjrt.so")
    m = types.ModuleType("antenv.axon_hooks")
    m.get_axon_ntff_profile_hook = lambda: hook
    m.set_axon_ntff_profile_hook = lambda h: None
    sys.modules["antenv.axon_hooks"] = m
    antenv.axon_hooks = m


def _wipe_compile_cache():
    """The neuron compile cache keys on HLO, which does not include the
    embedded Bass program — a previous build with the same I/O interface
    would be served stale. Wipe it so this build's NEFF is the one run."""
    import glob as _glob
    import shutil

    for pat in ("/root/.neuron-compile-cache", "/tmp/neuron-compile-cache-uid*"):
        for p in _glob.glob(pat):
            shutil.rmtree(p, ignore_errors=True)


def kernel(Q, K, V, Wq, Wk, wv, valid_lens):
    global LAST_EXEC_NS
    import ml_dtypes
    from concourse.bass_utils import run_bass_kernel_spmd

    _wipe_compile_cache()

    bfnp = ml_dtypes.bfloat16
    Q = np.asarray(Q, dtype=np.float32)
    K = np.asarray(K, dtype=np.float32)
    V = np.asarray(V, dtype=np.float32)
    Wq = np.asarray(Wq, dtype=np.float32)
    Wk = np.asarray(Wk, dtype=np.float32)
    wv = np.asarray(wv, dtype=np.float32)

    L, L2, KPV, CH, GB = _plan(valid_lens)
    nc = _build_program(L, L2, KPV, CH, GB)

    # shared tensors
    kt = np.ascontiguousarray(
        np.concatenate([K[b, : L2[b], :] for b in range(B)], axis=0).T
    ).astype(bfnp)
    v16 = np.ascontiguousarray(
        np.concatenate([V[b, : KPV[b], :] for b in range(B)], axis=0)
    ).astype(bfnp)
    oneh3 = np.zeros((H, SUBQ, SUBQ), dtype=bfnp)
    oneh3[:, np.arange(SUBQ), np.arange(SUBQ)] = wv[:, None].astype(bfnp)
    oneh = oneh3.reshape(H, SUBQ * SUBQ)
    eye = np.eye(SUBQ, dtype=bfnp)

    in_maps = []
    for c in range(NCORES):
        qloc = np.concatenate(
            [Q[b, c * QPC : (c + 1) * QPC, :] for b in range(B)], axis=0
        )  # (256, 512)
        in_maps.append(
            {
                "qt": np.ascontiguousarray(qloc.T).astype(bfnp),
                "kt": kt,
                "v": v16,
                "wq": Wq.astype(bfnp),
                "wk": Wk.astype(bfnp),
                "oneh": oneh,
                "eye": eye,
            }
        )

    trace = os.environ.get("KERNEL_PROFILE", "0") == "1"
    runs = int(os.environ.get("KERNEL_RUNS", "1"))
    if trace:
        _install_profile_hook()
    res = run_bass_kernel_spmd(nc, in_maps, list(range(NCORES)), trace=trace)
    LAST_EXEC_NS = res.exec_time_ns
    LAST_RESULT["res"] = res
    LAST_RESULT["times"] = [res.exec_time_ns]
    for _ in range(runs - 1):
        r2 = run_bass_kernel_spmd(nc, in_maps, list(range(NCORES)), trace=trace)
        LAST_RESULT["times"].append(r2.exec_time_ns)
        if r2.exec_time_ns and (not LAST_EXEC_NS or r2.exec_time_ns < LAST_EXEC_NS):
            LAST_EXEC_NS = r2.exec_time_ns
            LAST_RESULT["res"] = r2
            res = r2

    out = np.empty((B, NQ, DV), dtype=np.float32)
    for c in range(NCORES):
        o = np.asarray(res.results[c]["out"])
        for b in range(B):
            out[b, c * QPC : (c + 1) * QPC, :] = o[b * QPC : (b + 1) * QPC, :]
    return out


# revision 27
# speedup vs baseline: 1.1790x; 1.1790x over previous
import os
import sys

import numpy as np

sys.path.insert(0, "/opt/trn_rl_repo")

# Problem constants (nn_AdditiveAttention): hardcoded per spec.
B, NQ, NK, D, DV, H = 4, 512, 512, 512, 512, 128
NCORES = 8
QPC = NQ // NCORES  # queries contributed by each batch to each core (64)
SMAX = 7168         # max s/t pipeline tile free dim (per partition)
SUBQ = 64           # queries per softmax sub-group
WARM_MM = False     # emit HAM-warmer dummy matmuls

LAST_EXEC_NS = None
LAST_RESULT = {}


def _plan(valid_lens):
    L = [int(x) for x in np.asarray(valid_lens).reshape(-1)]
    L2 = [min(NK, -(-l // 2) * 2) for l in L]       # add/tanh/scores extent
    KPV = [min(NK, -(-l // 128) * 128) for l in L]  # PV (128-aligned) extent
    # Per batch: sub-groups of SUBQ queries, each a list of (chunk, fused).
    # The cheapest batch (smallest L2) is offloaded to GPSIMD as whole-chunk
    # tensor_tensor broadcast adds, if small enough to fit its slow rate.
    GB = -1  # gpsimd TT offload disabled: walrus re-engines it onto DVE
    CH = {}
    for b in range(B):
        c = 32
        while c * L2[b] > SMAX:
            c //= 2
        sgs = []
        for sg in range(QPC // SUBQ):
            specs = []
            left = SUBQ
            while left > 0:
                step = min(c, left)
                specs.append((step, False))
                left -= step
            sgs.append(specs)
        CH[b] = sgs
    return L, L2, KPV, CH, GB


def _build_program(L, L2, KPV, CH, GB):
    """Build the SPMD Bass program. All cores run this one program;
    per-core data differences come only through in_maps."""
    import concourse.bacc as bacc
    import concourse.mybir as mybir
    from concourse.tile import TileContext

    f32 = mybir.dt.float32
    bf16 = mybir.dt.bfloat16
    OFF2 = np.concatenate([[0], np.cumsum(L2)]).astype(int)
    OFFV = np.concatenate([[0], np.cumsum(KPV)]).astype(int)
    KSUM2 = int(OFF2[-1])
    KSUMV = int(OFFV[-1])
    NQL = B * QPC  # local queries per core (256)

    nc = bacc.Bacc("TRN2", target_bir_lowering=False, debug=False)

    qt_d = nc.dram_tensor("qt", [D, NQL], bf16, kind="ExternalInput")
    kt_d = nc.dram_tensor("kt", [D, KSUM2], bf16, kind="ExternalInput")
    v_d = nc.dram_tensor("v", [KSUMV, DV], bf16, kind="ExternalInput")
    wq_d = nc.dram_tensor("wq", [D, H], bf16, kind="ExternalInput")
    wk_d = nc.dram_tensor("wk", [D, H], bf16, kind="ExternalInput")
    oneh_d = nc.dram_tensor("oneh", [H, SUBQ * SUBQ], bf16, kind="ExternalInput")
    eye_d = nc.dram_tensor("eye", [SUBQ, SUBQ], bf16, kind="ExternalInput")
    out_d = nc.dram_tensor("out", [NQL, DV], f32, kind="ExternalOutput")

    Tanh = mybir.ActivationFunctionType.Tanh
    Exp = mybir.ActivationFunctionType.Exp
    Copy = mybir.ActivationFunctionType.Copy
    AX = mybir.AxisListType.X

    with TileContext(nc) as tc:
        with (
            tc.tile_pool(name="const", bufs=1) as cpool,
            tc.tile_pool(name="proj", bufs=1) as projpool,
            tc.tile_pool(name="s", bufs=3) as spool,
            tc.tile_pool(name="t", bufs=3) as tpool,
            tc.tile_pool(name="p", bufs=2) as ppool,
            tc.tile_pool(name="stat", bufs=6) as statpool,
            tc.tile_pool(name="osb", bufs=2) as opool,
        ):
            # ---- load constants (critical-path DMAs first, split across the
            # sync and gpsimd queues; small per-chunk loads are combined into
            # single DMAs; V tiles are loaded later, inside the main loop)
            kt_sb = [cpool.tile([128, KSUM2], bf16, tag=f"kt{i}", name=f"kt{i}") for i in range(4)]
            wkb = cpool.tile([128, 4 * H], bf16, tag="wkb")
            wqb = cpool.tile([128, 4 * H], bf16, tag="wqb")
            qtb = cpool.tile([128, 4 * NQL], bf16, tag="qtb")
            wk_sb = [wkb[:, i * H : (i + 1) * H] for i in range(4)]
            wq_sb = [wqb[:, i * H : (i + 1) * H] for i in range(4)]
            qt_sb = [qtb[:, i * NQL : (i + 1) * NQL] for i in range(4)]
            for i in range(4):
                eng = nc.sync if i % 2 == 0 else nc.gpsimd
                eng.dma_start(kt_sb[i][:], kt_d.rearrange("(n p) m -> n p m", p=128)[i])
            nc.sync.dma_start(wkb[:, :].rearrange("p (n m) -> p n m", n=4), wk_d.rearrange("(n p) m -> p n m", p=128))
            nc.gpsimd.dma_start(qtb[:, :].rearrange("p (n m) -> p n m", n=4), qt_d.rearrange("(n p) m -> p n m", p=128))
            nc.sync.dma_start(wqb[:, :].rearrange("p (n m) -> p n m", n=4), wq_d.rearrange("(n p) m -> p n m", p=128))
            oneh_sb = cpool.tile([128, SUBQ * SUBQ], bf16, tag="oneh")
            nc.gpsimd.dma_start(oneh_sb[:], oneh_d[:])
            eye_sb = cpool.tile([SUBQ, SUBQ], bf16, tag="eye")
            nc.sync.dma_start(eye_sb[:], eye_d[:])
            v_sb = [cpool.tile([128, DV], bf16, tag=f"v{i}", name=f"v{i}") for i in range(KSUMV // 128)]

            def load_v():
                for i in range(KSUMV // 128):
                    nc.sync.dma_start(
                        v_sb[i][:], v_d.rearrange("(n p) m -> n p m", p=128)[i]
                    )

            # ---- projections (bf16 in, f32 psum; QpT f32 / KpT bf16 out)
            qp_sb = projpool.tile([128, NQL], f32, tag="qp")
            kp_sb = [
                projpool.tile(
                    [128, L2[b]], f32 if b == GB else bf16,
                    tag=f"kp{b}", name=f"kp{b}",
                )
                for b in range(B)
            ]
            first_b = next(b for b in range(B) if b != GB)
            border = [first_b, -1] + [b for b in range(B) if b not in (first_b, -1, GB)] + ([GB] if GB >= 0 else [])
            with tc.tile_pool(name="pps", bufs=2, space="PSUM") as projps:
                for pb in border:
                    if pb == -1:
                        qp_ps = projps.tile([128, 512], f32, tag="projps", name="qp_ps")
                        for dc in range(4):
                            nc.tensor.matmul(
                                qp_ps[:, :NQL], wq_sb[dc][:], qt_sb[dc][:],
                                start=(dc == 0), stop=(dc == 3),
                            )
                        nc.scalar.copy(qp_sb[:], qp_ps[:, :NQL])
                        continue
                    b = pb
                    c0 = int(OFF2[b])
                    cw = L2[b]
                    kp_ps = projps.tile([128, 512], f32, tag="projps", name="kp_ps")
                    for dc in range(4):
                        nc.tensor.matmul(
                            kp_ps[:, :cw], wk_sb[dc][:], kt_sb[dc][:, c0 : c0 + cw],
                            start=(dc == 0), stop=(dc == 3),
                        )
                    nc.scalar.copy(kp_sb[b][:, :], kp_ps[:, :cw])

            # ---- main phase: one 32-query sub-group at a time, each with its
            # own PSUM tile; softmax/PV of sub-group g is split into 2 stages
            # drip-fed between later chunks so no engine stalls on the chain.
            with (
                tc.tile_pool(name="sps", bufs=3, space="PSUM") as scorps,
                tc.tile_pool(name="ops", bufs=2, space="PSUM") as ops,
                tc.tile_pool(name="tps", bufs=2, space="PSUM") as tps,
                tc.tile_pool(name="wps", bufs=1, space="PSUM") as wps,
            ):
                warm_ps = wps.tile([1, 8], f32, tag="warm")
                pending = []

                def softmax_stages(b, sg, sc_ps):
                    """Softmax + P@V for sub-group (b, sg) as 2 drip stages."""
                    kpadv = KPV[b]
                    koffv = int(OFFV[b])
                    lb = L[b]
                    r0 = b * QPC + sg * SUBQ  # output row base
                    box = {}

                    def s1():
                        nmx = statpool.tile([128, 1], f32, tag="nmx", name="nmx")
                        nc.vector.reduce_max(
                            nmx[:SUBQ, :], sc_ps[:SUBQ, :lb], axis=AX, negate=True
                        )
                        p_t = ppool.tile([SUBQ, 512], bf16, tag="p", name="p_t")
                        ssum = statpool.tile([128, 1], f32, tag="ssum", name="ssum")
                        nc.scalar.activation(
                            p_t[:, :lb], sc_ps[:SUBQ, :lb], Exp,
                            bias=nmx[:SUBQ, :], accum_out=ssum[:SUBQ, :],
                        )
                        if lb < kpadv:
                            nc.gpsimd.memset(p_t[:, lb:kpadv], 0.0)
                        box["p_t"] = p_t
                        box["ssum"] = ssum

                    def s2():
                        rs = statpool.tile([128, 1], f32, tag="rs", name="rs")
                        nc.vector.reciprocal(rs[:SUBQ, :], box["ssum"][:SUBQ, :])
                        o_ps = ops.tile([SUBQ, DV], f32, tag="ops", name="o_ps")
                        for kc in range(kpadv // 128):
                            wt_ps = tps.tile([128, SUBQ], bf16, tag="wtps", name="wt_ps")
                            nc.tensor.transpose(
                                wt_ps[:],
                                box["p_t"][:, kc * 128 : (kc + 1) * 128],
                                eye_sb[:],
                            )
                            wt_sb = statpool.tile([128, SUBQ], bf16, tag="wtsb", name="wt_sb")
                            nc.vector.tensor_copy(wt_sb[:], wt_ps[:])
                            nc.tensor.matmul(
                                o_ps[:],
                                wt_sb[:],
                                v_sb[(koffv + kc * 128) // 128][:],
                                start=(kc == 0), stop=(kc == kpadv // 128 - 1),
                            )
                        o_sb = opool.tile([SUBQ, DV], f32, tag="osb", name="o_sb")
                        nc.scalar.activation(
                            o_sb[:], o_ps[:], Copy, scale=rs[:SUBQ, :]
                        )
                        nc.sync.dma_start(out_d[r0 : r0 + SUBQ, :], o_sb[:])

                    return [s1, s2]

                # GPSIMD batch: emit all broadcast-add chunks up front;
                # their tanh+matmul consumption blocks drip into the stream.
                GCH = 8  # queries per gpsimd chunk
                g_s = []
                if GB >= 0:
                    gkpad = L2[GB]
                    for c in range(QPC // GCH):
                        s_t = spool.tile(
                            [128, GCH * gkpad], f32, tag=f"gs{c}", name="g_s_t"
                        )
                        kp_b = kp_sb[GB][:, :].rearrange(
                            "p (o k) -> p o k", o=1
                        ).broadcast_to((128, GCH, gkpad))
                        q0 = GB * QPC + c * GCH
                        qp_b = qp_sb[:, q0 : q0 + GCH].rearrange(
                            "p (c o) -> p c o", o=1
                        ).broadcast_to((128, GCH, gkpad))
                        nc.gpsimd.tensor_tensor(
                            out=s_t[:, :].rearrange("p (c k) -> p c k", c=GCH),
                            in0=kp_b, in1=qp_b, op=mybir.AluOpType.add,
                        )
                        g_s.append(s_t)

                g_sc = {}

                def g_block(c):
                    """Consume gpsimd chunk c: tanh + score matmuls."""
                    gkpad = L2[GB]
                    if c == 0:
                        g_sc[0] = scorps.tile(
                            [SUBQ, gkpad], f32, tag="scores", name="sc_ps"
                        )
                    sc_ps = g_sc[0]
                    t_t = tpool.tile([128, SMAX], bf16, tag="t", name="t_t")
                    nc.scalar.activation(
                        t_t[:, : GCH * gkpad], g_s[c][:, :], Tanh
                    )
                    for j in range(GCH):
                        jj = c * GCH + j
                        nc.tensor.matmul(
                            sc_ps[:SUBQ, :gkpad],
                            oneh_sb[:, jj * SUBQ : (jj + 1) * SUBQ],
                            t_t[:, j * gkpad : j * gkpad + gkpad],
                            start=(jj == 0), stop=(jj == SUBQ - 1),
                        )
                    if jj == SUBQ - 1:
                        pending.extend(softmax_stages(GB, 0, sc_ps))

                if GB >= 0:
                    pending.extend(
                        (lambda cc: (lambda: g_block(cc)))(c)
                        for c in range(QPC // GCH)
                    )

                for b in range(B):
                    if b == GB:
                        continue
                    kpad = L2[b]
                    for sg in range(QPC // SUBQ):
                        sc_ps = scorps.tile(
                            [SUBQ, kpad], f32, tag="scores", name="sc_ps"
                        )
                        qbase = 0
                        for chunk, fused in CH[b][sg]:
                            t_t = tpool.tile([128, SMAX], bf16, tag="t", name="t_t")
                            if fused:
                                for j in range(chunk):
                                    q = b * QPC + sg * SUBQ + qbase + j
                                    nc.scalar.activation(
                                        t_t[:, j * kpad : (j + 1) * kpad],
                                        kp_sb[b][:, :], Tanh,
                                        bias=qp_sb[:, q : q + 1],
                                    )
                            else:
                                s_t = spool.tile([128, SMAX], bf16, tag="s", name="s_t")
                                for j in range(chunk):
                                    q = b * QPC + sg * SUBQ + qbase + j
                                    nc.vector.tensor_scalar_add(
                                        s_t[:, j * kpad : (j + 1) * kpad],
                                        kp_sb[b][:, :],
                                        qp_sb[:, q : q + 1],
                                    )
                                if WARM_MM:
                                    nc.tensor.matmul(
                                        warm_ps[:, :1],
                                        oneh_sb[:, 0:1],
                                        s_t[:, 0:1],
                                        start=True, stop=True,
                                    )
                                fd = chunk * kpad
                                nc.scalar.activation(
                                    t_t[:, :fd], s_t[:, :fd], Tanh
                                )
                            for j in range(chunk):
                                jj = sg * SUBQ % SUBQ + qbase + j  # within group
                                nc.tensor.matmul(
                                    sc_ps[:SUBQ, :kpad],
                                    oneh_sb[:, jj * SUBQ : (jj + 1) * SUBQ],
                                    t_t[:, j * kpad : j * kpad + kpad],
                                    start=(jj == 0), stop=(jj == SUBQ - 1),
                                )
                            qbase += chunk
                            if b == (1 if GB != 1 else 2) and sg == 0 and qbase == chunk:
                                load_v()
                            if pending:
                                pending.pop(0)()
                        pending.extend(softmax_stages(b, sg, sc_ps))
                while pending:
                    pending.pop(0)()

    nc.compile()
    return nc


def _install_profile_hook():
    """Register the NTFF profile hook that this container's antenv lacks,
    so run_bass_kernel_spmd(trace=True) can report exec_time_ns."""
    import types

    import antenv

    try:
        import antenv.axon_hooks  # noqa: F401
        return
    except ImportError:
        pass
    try:
        from trn_agent_boot.trn_boot import _ntff_profile_via_ctypes
    except ImportError:
        return
    hook = _ntff_profile_via_ctypes("/opt/axon/libaxon_pjrt.so")
    m = types.ModuleType("antenv.axon_hooks")
    m.get_axon_ntff_profile_hook = lambda: hook
    m.set_axon_ntff_profile_hook = lambda h: None
    sys.modules["antenv.axon_hooks"] = m
    antenv.axon_hooks = m


def _wipe_compile_cache():
    """The neuron compile cache keys on HLO, which does not include the
    embedded Bass program — a previous build with the same I/O interface
    would be served stale. Wipe it so this build's NEFF is the one run."""
    import glob as _glob
    import shutil

    for pat in ("/root/.neuron-compile-cache", "/tmp/neuron-compile-cache-uid*"):
        for p in _glob.glob(pat):
            shutil.rmtree(p, ignore_errors=True)


def kernel(Q, K, V, Wq, Wk, wv, valid_lens):
    global LAST_EXEC_NS
    import ml_dtypes
    from concourse.bass_utils import run_bass_kernel_spmd

    _wipe_compile_cache()

    bfnp = ml_dtypes.bfloat16
    Q = np.asarray(Q, dtype=np.float32)
    K = np.asarray(K, dtype=np.float32)
    V = np.asarray(V, dtype=np.float32)
    Wq = np.asarray(Wq, dtype=np.float32)
    Wk = np.asarray(Wk, dtype=np.float32)
    wv = np.asarray(wv, dtype=np.float32)

    L, L2, KPV, CH, GB = _plan(valid_lens)
    nc = _build_program(L, L2, KPV, CH, GB)

    # shared tensors
    kt = np.ascontiguousarray(
        np.concatenate([K[b, : L2[b], :] for b in range(B)], axis=0).T
    ).astype(bfnp)
    v16 = np.ascontiguousarray(
        np.concatenate([V[b, : KPV[b], :] for b in range(B)], axis=0)
    ).astype(bfnp)
    oneh3 = np.zeros((H, SUBQ, SUBQ), dtype=bfnp)
    oneh3[:, np.arange(SUBQ), np.arange(SUBQ)] = wv[:, None].astype(bfnp)
    oneh = oneh3.reshape(H, SUBQ * SUBQ)
    eye = np.eye(SUBQ, dtype=bfnp)

    in_maps = []
    for c in range(NCORES):
        qloc = np.concatenate(
            [Q[b, c * QPC : (c + 1) * QPC, :] for b in range(B)], axis=0
        )  # (256, 512)
        in_maps.append(
            {
                "qt": np.ascontiguousarray(qloc.T).astype(bfnp),
                "kt": kt,
                "v": v16,
                "wq": Wq.astype(bfnp),
                "wk": Wk.astype(bfnp),
                "oneh": oneh,
                "eye": eye,
            }
        )

    trace = os.environ.get("KERNEL_PROFILE", "0") == "1"
    runs = int(os.environ.get("KERNEL_RUNS", "1"))
    if trace:
        _install_profile_hook()
    res = run_bass_kernel_spmd(nc, in_maps, list(range(NCORES)), trace=trace)
    LAST_EXEC_NS = res.exec_time_ns
    LAST_RESULT["res"] = res
    LAST_RESULT["times"] = [res.exec_time_ns]
    for _ in range(runs - 1):
        r2 = run_bass_kernel_spmd(nc, in_maps, list(range(NCORES)), trace=trace)
        LAST_RESULT["times"].append(r2.exec_time_ns)
        if r2.exec_time_ns and (not LAST_EXEC_NS or r2.exec_time_ns < LAST_EXEC_NS):
            LAST_EXEC_NS = r2.exec_time_ns
            LAST_RESULT["res"] = r2
            res = r2

    out = np.empty((B, NQ, DV), dtype=np.float32)
    for c in range(NCORES):
        o = np.asarray(res.results[c]["out"])
        for b in range(B):
            out[b, c * QPC : (c + 1) * QPC, :] = o[b * QPC : (b + 1) * QPC, :]
    return out
